# revision 22
# baseline (speedup 1.0000x reference)
"""GNN attention kernel for Trainium2, SPMD over 8 NeuronCores.

Reference computation (per batch b, head h):
    Xp   = X @ Wp[h] + bp[h]                  [N, DH]
    Xc   = Xp @ C[h].T                        [N, DH]
    S    = Xc @ Xp.T                          [N, N]
    attn = tanh(A * S) = A * tanh(S)          (A is binary, tanh(0)=0)
    Xh   = attn @ Xp                          [N, DH]
    out  = relu(concat_h Xh)                  [N, DOUT]

Sharding: data-parallel over batch B=32 -> 4 batches per core. No collectives.

Per-core dataflow (bf16 on PE, f32 PSUM accumulate):
  - X[b], A[b] DMA'd natural f32 (HWDGE); transposed on PE, cast to bf16 on
    the PSUM drain: XT [d, n], AT [m, n].
  - Xp  [m, (h k)] = XT.T @ Wp        (aggregate rhs)
  - XpT [(2h k), q, n] = Wp_pair.T @ XT   (scores lhsT source)
  - XcT [(j|0), h, n] = Cpad[h].T @ XpT — Cpad is C^T embedded in a zero
    padded [128,128] block at rows/cols (h%2)*64, so each head's XcT lands
    on its own partition rows with zeros elsewhere; K=128 matmuls always
    read operands at partition base 0 (HW mishandles base alternation).
  - Scores T_h[m, n] = xpt-block.T @ xct[h]: cross-head terms hit zero rows.
  - tanh on ACT (PSUM f32 -> SBUF bf16, fused drain); mask-mul with AT
    split across DVE and GPSIMD (bf16).
  - Aggregate: lhsT = attnT tile, rhs = Xp -> Xh[n, k] in PSUM, ReLU to
    f32 out tile on DVE, DMA out.
"""

import os
import sys
import types
import numpy as np

import concourse.bass as bass
import concourse.tile as tile
from concourse import bacc, mybir
from concourse.bass_utils import run_bass_kernel_spmd
from concourse.masks import make_identity


def _install_ntff_hook():
    """The image's ``antenv`` lacks ``axon_hooks``; shim it so
    ``run_bass_kernel_spmd(trace=True)`` can capture NTFF profiles through
    the ctypes hook from ``trn_agent_boot``. Degrades silently."""
    if "antenv.axon_hooks" in sys.modules:
        return
    try:
        import antenv  # noqa: F401

        mod = types.ModuleType("antenv.axon_hooks")
        mod._hook = None

        def set_axon_ntff_profile_hook(h):
            mod._hook = h

        def get_axon_ntff_profile_hook():
            return mod._hook

        mod.set_axon_ntff_profile_hook = set_axon_ntff_profile_hook
        mod.get_axon_ntff_profile_hook = get_axon_ntff_profile_hook
        sys.modules["antenv.axon_hooks"] = mod
        from trn_agent_boot.trn_boot import _ntff_profile_via_ctypes

        hook = _ntff_profile_via_ctypes("/opt/axon/libaxon_pjrt.so")
        if hook is not None:
            mod._hook = hook
    except Exception:
        pass


_install_ntff_hook()

B, N, DIN, DOUT, H, DH = 32, 1024, 512, 512, 8, 64
NCORES = 8
BS = B // NCORES          # 4 batches per core
NCH = N // 128            # 8 n/m chunks of 128
DT = DIN // 128           # 4 d tiles
PAIRS = H // 2            # 4 head pairs

F32 = mybir.dt.float32
BF16 = mybir.dt.bfloat16
AF = mybir.ActivationFunctionType

LAST_EXEC_NS = None
LAST_TRACE_DIR = None


def _build(with_bias: bool, stage: int = 4, n_batch: int = BS):
    nc = bacc.Bacc("TRN2", target_bir_lowering=False, debug=False,
                   num_devices=NCORES)
    X = nc.dram_tensor("X", [BS, N, DIN], F32, kind="ExternalInput").ap()
    A = nc.dram_tensor("A", [BS, N, N], F32, kind="ExternalInput").ap()
    Wp = nc.dram_tensor("Wp", [H, DIN, DH], F32, kind="ExternalInput").ap()
    C = nc.dram_tensor("C", [H, DH, DH], F32, kind="ExternalInput").ap()
    bp = None
    if with_bias:
        bp = nc.dram_tensor("bp", [H, DH], F32, kind="ExternalInput").ap()
    OUT = nc.dram_tensor("out", [BS, N, DOUT], F32, kind="ExternalOutput").ap()

    with tile.TileContext(nc) as tc:
        with (
            tc.tile_pool(name="singles", bufs=1) as singles,
            tc.tile_pool(name="xa", bufs=1) as xa,
            tc.tile_pool(name="bpool", bufs=2) as bpool,      # xp (dbl buf)
            tc.tile_pool(name="b1pool", bufs=1) as b1pool,    # xt/at/outsb/...
            tc.tile_pool(name="attnp", bufs=3) as attnp,      # attnT per head
            tc.tile_pool(name="atp", bufs=2) as atp,          # A^T (dbl buf)
            tc.tile_pool(name="xhtp", bufs=2) as xhtp,        # XhT pair tiles
            tc.tile_pool(name="psT", bufs=2, space="PSUM") as psT,       # 4 bank
            tc.tile_pool(name="psProj", bufs=2, space="PSUM") as psProj,  # 2 bank
            tc.tile_pool(name="psXh", bufs=2, space="PSUM") as psXh,     # 2 bank
        ):
            # ---- one-time setup ----
            ident = singles.tile([128, 128], F32, name="ident")
            make_identity(nc, ident)
            identb = singles.tile([128, 128], BF16, name="identb")
            make_identity(nc, identb)

            # Wp: [H, DIN, DH] -> stage [128, t, h, k] f32 -> bf16
            wp_stage = xa.tile([128, DT, H, DH], F32, tag="anat", name="wp_stage")
            wp_r = Wp.rearrange("h (t p) k -> p t h k", p=128)
            for t in range(DT):
                nc.gpsimd.dma_start(out=wp_stage[:, t, :, :], in_=wp_r[:, t, :, :])
            wp_sb = singles.tile([128, DT, H, DH], BF16, name="wp_sb")
            nc.vector.tensor_copy(wp_sb, wp_stage)

            # Cpad[*, h, :]: C^T[h] embedded at rows/cols (h%2)*64 of a zeroed
            # [128,128] block. Built via PE transpose (PSUM partitions 0:64),
            # a DMA partition-duplicate, then parity-placed copies.
            c_stage = xa.tile([DH, H, DH], F32, tag="xnat", name="c_stage")
            nc.gpsimd.dma_start(out=c_stage, in_=C.rearrange("h j k -> j h k"))
            ps_ct = psProj.tile([128, 512], F32, tag="proj", name="ps_ct")
            for h in range(H):
                nc.tensor.transpose(ps_ct[:DH, h * DH:(h + 1) * DH],
                                    c_stage[:, h, :], ident[:DH, :DH])
            ct_sb = singles.tile([128, H, DH], BF16, name="ct_sb")
            nc.vector.tensor_copy(
                ct_sb[:DH],
                ps_ct[:DH, :H * DH].rearrange("p (h j) -> p h j", h=H))
            nc.gpsimd.dma_start(out=ct_sb[DH:], in_=ct_sb[:DH])
            ctpad = singles.tile([128, H, 128], BF16, name="ctpad")
            nc.vector.memset(ctpad, 0.0)
            for h in range(H):
                u = h % 2
                sl = slice(u * DH, (u + 1) * DH)
                nc.vector.tensor_copy(ctpad[sl, h, sl], ct_sb[sl, h, :])

            bias_sb = None
            bias_pcol = None
            if with_bias:
                # bp broadcast to all partitions: [128, (h k)] f32 (Xp layout)
                bias_sb = singles.tile([128, H * DH], F32, name="bias_sb")
                bp_flat = bp.rearrange("h k -> (h k)")
                bcast = bass.AP(tensor=bp_flat.tensor, offset=bp_flat.offset,
                                ap=[[0, 128]] + list(bp_flat.ap))
                nc.gpsimd.dma_start(out=bias_sb, in_=bcast)
                # bp along partitions for XpT layout: [:, q] = bp_flat[128q:+128]
                bias_pcol = singles.tile([128, PAIRS], F32, name="bias_pcol")
                pcol = bass.AP(tensor=bp_flat.tensor, offset=bp_flat.offset,
                               ap=[[1, 128], [128, PAIRS]])
                nc.gpsimd.dma_start(out=bias_pcol, in_=pcol)

            # ---- per-batch pipeline ----
            for b in range(n_batch):
                # loads (natural layout, f32) on HWDGE queues
                x_nat = xa.tile([128, NCH, DIN], F32, tag="xnat", name="x_nat")
                a_nat = xa.tile([128, NCH, N], F32, tag="anat", name="a_nat")
                for j in range(NCH):
                    nc.sync.dma_start(out=x_nat[:, j, :],
                                      in_=X[b, j * 128:(j + 1) * 128, :])
                for j in range(NCH):
                    nc.sync.dma_start(out=a_nat[:, j, :],
                                      in_=A[b, j * 128:(j + 1) * 128, :])

                # XT [d, (t n)]: PE transpose of x_nat, drained in 512 halves
                xt_sb = b1pool.tile([128, DT, N], BF16, tag="xt", name="xt_sb")
                for t in range(DT):
                    for nh in range(2):
                        ps_xt = psProj.tile([128, 512], F32, tag="proj",
                                            name="ps_xt")
                        for j2 in range(4):
                            j = nh * 4 + j2
                            nc.tensor.transpose(
                                ps_xt[:, j2 * 128:(j2 + 1) * 128],
                                x_nat[:, j, t * 128:(t + 1) * 128], ident)
                        nc.vector.tensor_copy(
                            xt_sb[:, t, nh * 512:(nh + 1) * 512], ps_xt)

                # AT [m, (i n)]: PE transpose of a_nat (f32), cast to bf16
                # on the drain; drains split DVE/ACT
                at_sb = atp.tile([128, NCH, N], BF16, tag="at", name="at_sb")
                for i in range(NCH):
                    for nh in range(2):
                        ps_at = psProj.tile([128, 512], F32, tag="proj",
                                            name="ps_at")
                        for j2 in range(4):
                            j = nh * 4 + j2
                            nc.tensor.transpose(
                                ps_at[:, j2 * 128:(j2 + 1) * 128],
                                a_nat[:, j, i * 128:(i + 1) * 128], ident)
                        dst = at_sb[:, i, nh * 512:(nh + 1) * 512]
                        if nh == 0:
                            nc.vector.tensor_copy(dst, ps_at)
                        else:
                            nc.scalar.activation(dst, ps_at, AF.Copy)

                if stage < 1.2:
                    continue

                # Xp (all heads) [m, (h k)] bf16: aggregate rhs
                xp_sb = bpool.tile([128, NCH, H * DH], BF16, tag="xp",
                                   name="xp_sb")
                for j in range(NCH):
                    ps_xp = psProj.tile([128, H * DH], F32, tag="proj",
                                        name="ps_xp")
                    for t in range(DT):
                        nc.tensor.matmul(ps_xp,
                                         xt_sb[:, t, j * 128:(j + 1) * 128],
                                         wp_sb[:, t, :, :],
                                         start=(t == 0), stop=(t == DT - 1))
                    if with_bias:
                        nc.vector.tensor_add(ps_xp, ps_xp, bias_sb)
                    nc.vector.tensor_copy(xp_sb[:, j, :], ps_xp)

                if stage < 1.4:
                    continue

                # XpT [(2h k), q, n] = Wp_pair.T @ XT
                xpt_sb = b1pool.tile([128, PAIRS, N], BF16, tag="xpt",
                                     name="xpt_sb")
                for q in range(PAIRS):
                    for nh in range(2):
                        ps_xpt = psProj.tile([128, 512], F32, tag="proj",
                                             name="ps_xpt")
                        for t in range(DT):
                            nc.tensor.matmul(
                                ps_xpt,
                                wp_sb[:, t, 2 * q:2 * q + 2, :],
                                xt_sb[:, t, nh * 512:(nh + 1) * 512],
                                start=(t == 0), stop=(t == DT - 1))
                        if with_bias:
                            nc.vector.tensor_scalar_add(
                                ps_xpt, ps_xpt, bias_pcol[:, q:q + 1])
                        nc.vector.tensor_copy(
                            xpt_sb[:, q, nh * 512:(nh + 1) * 512], ps_xpt)

                if stage < 1.6:
                    continue

                # XcT [(j|0), h, n] = Cpad[h].T @ XpT  (zero rows kill the
                # other head's contribution; drains copy the zeros too)
                xct_sb = b1pool.tile([128, H, N], BF16, tag="xct", name="xct_sb")
                for h in range(H):
                    q = h // 2
                    for nh in range(2):
                        ps_xct = psProj.tile([128, 512], F32, tag="proj",
                                             name="ps_xct")
                        nc.tensor.matmul(ps_xct, ctpad[:, h, :],
                                         xpt_sb[:, q, nh * 512:(nh + 1) * 512],
                                         start=True, stop=True)
                        dst = xct_sb[:, h, nh * 512:(nh + 1) * 512]
                        if nh == 0:
                            nc.vector.tensor_copy(dst, ps_xct)
                        else:
                            nc.scalar.activation(dst, ps_xct, AF.Copy)

                out_sb = b1pool.tile([128, NCH, DOUT], F32, tag="outsb",
                                     name="out_sb")

                if stage < 3:
                    continue
                for q in range(PAIRS):
                    # scores + tanh + mask, per head
                    attn_t = []
                    for u in range(2):
                        h = 2 * q + u
                        at_u = attnp.tile([128, NCH, N], BF16, tag="attnT",
                                          name="attn_t")
                        for i in range(NCH):
                            ps_t = psT.tile([128, N], F32, tag="T", name="ps_t")
                            for nh in range(2):
                                nc.tensor.matmul(
                                    ps_t[:, nh * 512:(nh + 1) * 512],
                                    xpt_sb[:, q, i * 128:(i + 1) * 128],
                                    xct_sb[:, h, nh * 512:(nh + 1) * 512],
                                    start=True, stop=True)
                            nc.scalar.activation(at_u[:, i, :], ps_t, AF.Tanh)
                            eng = nc.gpsimd if i % 4 == 3 else nc.vector
                            eng.tensor_mul(at_u[:, i, :], at_u[:, i, :],
                                           at_sb[:, i, :])
                        attn_t.append(at_u)

                    if stage < 4:
                        continue
                    # aggregate, transposed: XhT_pair = Xp_pair.T @ attnT
                    # (512-wide matmuls so weight loads pipeline; the off-head
                    # half of each PSUM result is junk). ReLU fuses into the
                    # bf16 drain; a [128,128] PE transpose per (q, j) restores
                    # the natural [n, k] layout for the output tile.
                    xht = xhtp.tile([128, N], BF16, tag="xht", name="xht")
                    for u in range(2):
                        usl = slice(u * DH, (u + 1) * DH)
                        for nh in range(2):
                            ps_xh = psXh.tile([128, 512], F32, tag="xh",
                                              name="ps_xh")
                            for i in range(NCH):
                                nc.tensor.matmul(
                                    ps_xh,
                                    xp_sb[:, i, q * 128:(q + 1) * 128],
                                    attn_t[u][:, i, nh * 512:(nh + 1) * 512],
                                    start=(i == 0), stop=(i == NCH - 1))
                            nc.vector.tensor_scalar_max(
                                xht[usl, nh * 512:(nh + 1) * 512],
                                ps_xh[usl, :], 0.0)
                    for jj in range(2):
                        ps_o = psProj.tile([128, 512], BF16, tag="proj",
                                           name="ps_o")
                        for j2 in range(4):
                            j = jj * 4 + j2
                            nc.tensor.transpose(
                                ps_o[:, j2 * 128:(j2 + 1) * 128],
                                xht[:, j * 128:(j + 1) * 128], identb)
                        nc.vector.tensor_copy(
                            out_sb[:, jj * 4:(jj + 1) * 4,
                                   q * 128:(q + 1) * 128],
                            ps_o.rearrange("p (j c) -> p j c", j=4))

                if stage >= 4:
                    for j in range(NCH):
                        nc.sync.dma_start(out=OUT[b, j * 128:(j + 1) * 128, :],
                                          in_=out_sb[:, j, :])

    nc.compile()
    return nc


_CACHED = {}


def _get_nc(with_bias: bool):
    if with_bias not in _CACHED:
        _CACHED[with_bias] = _build(with_bias)
    return _CACHED[with_bias]


def kernel(X, A, Wp, bp, C):
    global LAST_EXEC_NS, LAST_TRACE_DIR
    X = np.ascontiguousarray(np.asarray(X, dtype=np.float32))
    A = np.ascontiguousarray(np.asarray(A, dtype=np.float32))
    Wp = np.ascontiguousarray(np.asarray(Wp, dtype=np.float32))
    bp = np.ascontiguousarray(np.asarray(bp, dtype=np.float32))
    C = np.ascontiguousarray(np.asarray(C, dtype=np.float32))

    with_bias = bool(np.any(bp))
    nc = _get_nc(with_bias)

    in_maps = []
    for c in range(NCORES):
        m = {
            "X": X[c * BS:(c + 1) * BS],
            "A": A[c * BS:(c + 1) * BS],
            "Wp": Wp,
            "C": C,
        }
        if with_bias:
            m["bp"] = bp
        in_maps.append(m)

    trace = bool(int(os.environ.get("KERNEL_TRACE", "0")))
    res = run_bass_kernel_spmd(nc, in_maps, core_ids=list(range(NCORES)),
                               trace=trace)
    LAST_EXEC_NS = res.exec_time_ns
    if res.instructions_and_trace is not None:
        LAST_TRACE_DIR = res.instructions_and_trace[1]
    out = np.concatenate([res.results[c]["out"] for c in range(NCORES)], axis=0)
    return out.astype(np.float32)


# revision 23
# speedup vs baseline: 1.0633x; 1.0633x over previous
"""GNN attention kernel for Trainium2, SPMD over 8 NeuronCores.

Reference computation (per batch b, head h):
    Xp   = X @ Wp[h] + bp[h]                  [N, DH]
    Xc   = Xp @ C[h].T                        [N, DH]
    S    = Xc @ Xp.T                          [N, N]
    attn = tanh(A * S) = A * tanh(S)          (A is binary, tanh(0)=0)
    Xh   = attn @ Xp                          [N, DH]
    out  = relu(concat_h Xh)                  [N, DOUT]

Sharding: data-parallel over batch B=32 -> 4 batches per core. No collectives.

Per-core dataflow (bf16 on PE, f32 PSUM accumulate):
  - X[b], A[b] DMA'd natural f32 (HWDGE); transposed on PE, cast to bf16 on
    the PSUM drain: XT [d, n], AT [m, n].
  - Xp  [m, (h k)] = XT.T @ Wp        (aggregate rhs)
  - XpT [(2h k), q, n] = Wp_pair.T @ XT   (scores lhsT source)
  - XcT [(j|0), h, n] = Cpad[h].T @ XpT — Cpad is C^T embedded in a zero
    padded [128,128] block at rows/cols (h%2)*64, so each head's XcT lands
    on its own partition rows with zeros elsewhere; K=128 matmuls always
    read operands at partition base 0 (HW mishandles base alternation).
  - Scores T_h[m, n] = xpt-block.T @ xct[h]: cross-head terms hit zero rows.
  - tanh on ACT (PSUM f32 -> SBUF bf16, fused drain); mask-mul with AT
    split across DVE and GPSIMD (bf16).
  - Aggregate: lhsT = attnT tile, rhs = Xp -> Xh[n, k] in PSUM, ReLU to
    f32 out tile on DVE, DMA out.
"""

import os
import sys
import types
import numpy as np

import concourse.bass as bass
import concourse.tile as tile
from concourse import bacc, mybir
from concourse.bass_utils import run_bass_kernel_spmd
from concourse.masks import make_identity


def _install_ntff_hook():
    """The image's ``antenv`` lacks ``axon_hooks``; shim it so
    ``run_bass_kernel_spmd(trace=True)`` can capture NTFF profiles through
    the ctypes hook from ``trn_agent_boot``. Degrades silently."""
    if "antenv.axon_hooks" in sys.modules:
        return
    try:
        import antenv  # noqa: F401

        mod = types.ModuleType("antenv.axon_hooks")
        mod._hook = None

        def set_axon_ntff_profile_hook(h):
            mod._hook = h

        def get_axon_ntff_profile_hook():
            return mod._hook

        mod.set_axon_ntff_profile_hook = set_axon_ntff_profile_hook
        mod.get_axon_ntff_profile_hook = get_axon_ntff_profile_hook
        sys.modules["antenv.axon_hooks"] = mod
        from trn_agent_boot.trn_boot import _ntff_profile_via_ctypes

        hook = _ntff_profile_via_ctypes("/opt/axon/libaxon_pjrt.so")
        if hook is not None:
            mod._hook = hook
    except Exception:
        pass


_install_ntff_hook()

B, N, DIN, DOUT, H, DH = 32, 1024, 512, 512, 8, 64
NCORES = 8
BS = B // NCORES          # 4 batches per core
NCH = N // 128            # 8 n/m chunks of 128
DT = DIN // 128           # 4 d tiles
PAIRS = H // 2            # 4 head pairs

F32 = mybir.dt.float32
BF16 = mybir.dt.bfloat16
AF = mybir.ActivationFunctionType

LAST_EXEC_NS = None
LAST_TRACE_DIR = None


def _build(with_bias: bool, stage: int = 4, n_batch: int = BS):
    nc = bacc.Bacc("TRN2", target_bir_lowering=False, debug=False,
                   num_devices=NCORES)
    X = nc.dram_tensor("X", [BS, N, DIN], F32, kind="ExternalInput").ap()
    A = nc.dram_tensor("A", [BS, N, N], F32, kind="ExternalInput").ap()
    Wp = nc.dram_tensor("Wp", [H, DIN, DH], F32, kind="ExternalInput").ap()
    C = nc.dram_tensor("C", [H, DH, DH], F32, kind="ExternalInput").ap()
    bp = None
    if with_bias:
        bp = nc.dram_tensor("bp", [H, DH], F32, kind="ExternalInput").ap()
    OUT = nc.dram_tensor("out", [BS, N, DOUT], F32, kind="ExternalOutput").ap()

    with tile.TileContext(nc) as tc:
        with (
            tc.tile_pool(name="singles", bufs=1) as singles,
            tc.tile_pool(name="xa", bufs=1) as xa,
            tc.tile_pool(name="bpool", bufs=2) as bpool,      # xp (dbl buf)
            tc.tile_pool(name="b1pool", bufs=1) as b1pool,    # xt/at/outsb/...
            tc.tile_pool(name="attnp", bufs=3) as attnp,      # attnT per head
            tc.tile_pool(name="atp", bufs=2) as atp,          # A^T (dbl buf)
            tc.tile_pool(name="abp", bufs=2) as abp,          # A bf16 halves
            tc.tile_pool(name="psT", bufs=2, space="PSUM") as psT,       # 4 bank
            tc.tile_pool(name="psProj", bufs=3, space="PSUM") as psProj,  # 3 bank
            tc.tile_pool(name="psXh", bufs=1, space="PSUM") as psXh,     # 1 bank
        ):
            # ---- one-time setup ----
            ident = singles.tile([128, 128], F32, name="ident")
            make_identity(nc, ident)
            identb = singles.tile([128, 128], BF16, name="identb")
            make_identity(nc, identb)

            # Wp: [H, DIN, DH] -> stage [128, t, h, k] f32 -> bf16
            wp_stage = xa.tile([128, DT, H, DH], F32, tag="anat", name="wp_stage")
            wp_r = Wp.rearrange("h (t p) k -> p t h k", p=128)
            for t in range(DT):
                nc.gpsimd.dma_start(out=wp_stage[:, t, :, :], in_=wp_r[:, t, :, :])
            wp_sb = singles.tile([128, DT, H, DH], BF16, name="wp_sb")
            nc.vector.tensor_copy(wp_sb, wp_stage)

            # Cpad[*, h, :]: C^T[h] embedded at rows/cols (h%2)*64 of a zeroed
            # [128,128] block. Built via PE transpose (PSUM partitions 0:64),
            # a DMA partition-duplicate, then parity-placed copies.
            c_stage = xa.tile([DH, H, DH], F32, tag="xnat", name="c_stage")
            nc.gpsimd.dma_start(out=c_stage, in_=C.rearrange("h j k -> j h k"))
            ps_ct = psProj.tile([128, 512], F32, tag="proj", name="ps_ct")
            for h in range(H):
                nc.tensor.transpose(ps_ct[:DH, h * DH:(h + 1) * DH],
                                    c_stage[:, h, :], ident[:DH, :DH])
            ct_sb = singles.tile([128, H, DH], BF16, name="ct_sb")
            nc.vector.tensor_copy(
                ct_sb[:DH],
                ps_ct[:DH, :H * DH].rearrange("p (h j) -> p h j", h=H))
            nc.gpsimd.dma_start(out=ct_sb[DH:], in_=ct_sb[:DH])
            ctpad = singles.tile([128, H, 128], BF16, name="ctpad")
            nc.vector.memset(ctpad, 0.0)
            for h in range(H):
                u = h % 2
                sl = slice(u * DH, (u + 1) * DH)
                nc.vector.tensor_copy(ctpad[sl, h, sl], ct_sb[sl, h, :])

            bias_sb = None
            bias_pcol = None
            if with_bias:
                # bp broadcast to all partitions: [128, (h k)] f32 (Xp layout)
                bias_sb = singles.tile([128, H * DH], F32, name="bias_sb")
                bp_flat = bp.rearrange("h k -> (h k)")
                bcast = bass.AP(tensor=bp_flat.tensor, offset=bp_flat.offset,
                                ap=[[0, 128]] + list(bp_flat.ap))
                nc.gpsimd.dma_start(out=bias_sb, in_=bcast)
                # bp along partitions for XpT layout: [:, q] = bp_flat[128q:+128]
                bias_pcol = singles.tile([128, PAIRS], F32, name="bias_pcol")
                pcol = bass.AP(tensor=bp_flat.tensor, offset=bp_flat.offset,
                               ap=[[1, 128], [128, PAIRS]])
                nc.gpsimd.dma_start(out=bias_pcol, in_=pcol)

            # ---- per-batch pipeline ----
            for b in range(n_batch):
                # loads (natural layout, f32) on HWDGE queues; A arrives in
                # two j-half stages so the f32 staging tile stays small
                x_nat = xa.tile([128, NCH, DIN], F32, tag="xnat", name="x_nat")
                for j in range(NCH):
                    nc.sync.dma_start(out=x_nat[:, j, :],
                                      in_=X[b, j * 128:(j + 1) * 128, :])
                xb_sb = b1pool.tile([128, NCH, DIN], BF16, tag="xb", name="xb_sb")
                for j in range(NCH):
                    nc.vector.tensor_copy(xb_sb[:, j, :], x_nat[:, j, :])
                ab_sb = []
                for jh in range(2):
                    a_half = xa.tile([128, 4, N], F32, tag="anat", name="a_half")
                    for j2 in range(4):
                        j = jh * 4 + j2
                        nc.sync.dma_start(out=a_half[:, j2, :],
                                          in_=A[b, j * 128:(j + 1) * 128, :])
                    ab_h = abp.tile([128, 4, N], BF16, tag="ab", name="ab_h")
                    for j2 in range(4):
                        nc.vector.tensor_copy(ab_h[:, j2, :], a_half[:, j2, :])
                    ab_sb.append(ab_h)

                # XT [d, (t n)]: PE transpose of x_nat, drained in 512 halves
                xt_sb = b1pool.tile([128, DT, N], BF16, tag="xt", name="xt_sb")
                for t in range(DT):
                    for nh in range(2):
                        ps_xt = psProj.tile([128, 512], BF16, tag="proj",
                                            name="ps_xt")
                        for j2 in range(4):
                            j = nh * 4 + j2
                            nc.tensor.transpose(
                                ps_xt[:, j2 * 128:(j2 + 1) * 128],
                                xb_sb[:, j, t * 128:(t + 1) * 128], identb)
                        nc.vector.tensor_copy(
                            xt_sb[:, t, nh * 512:(nh + 1) * 512], ps_xt)

                # AT [m, (i n)]: PE transpose of bf16 A halves; drains
                # split DVE/ACT
                at_sb = atp.tile([128, NCH, N], BF16, tag="at", name="at_sb")
                for nh in range(2):
                    for i in range(NCH):
                        ps_at = psProj.tile([128, 512], BF16, tag="proj",
                                            name="ps_at")
                        for j2 in range(4):
                            nc.tensor.transpose(
                                ps_at[:, j2 * 128:(j2 + 1) * 128],
                                ab_sb[nh][:, j2, i * 128:(i + 1) * 128], identb)
                        dst = at_sb[:, i, nh * 512:(nh + 1) * 512]
                        if i % 2 == 0:
                            nc.vector.tensor_copy(dst, ps_at)
                        else:
                            nc.scalar.activation(dst, ps_at, AF.Copy)

                if stage < 1.2:
                    continue

                # Xp (all heads) [m, (h k)] bf16: aggregate rhs
                xp_sb = bpool.tile([128, NCH, H * DH], BF16, tag="xp",
                                   name="xp_sb")
                for j in range(NCH):
                    ps_xp = psProj.tile([128, H * DH], F32, tag="proj",
                                        name="ps_xp")
                    for t in range(DT):
                        nc.tensor.matmul(ps_xp,
                                         xt_sb[:, t, j * 128:(j + 1) * 128],
                                         wp_sb[:, t, :, :],
                                         start=(t == 0), stop=(t == DT - 1))
                    if with_bias:
                        nc.vector.tensor_add(ps_xp, ps_xp, bias_sb)
                    nc.vector.tensor_copy(xp_sb[:, j, :], ps_xp)

                if stage < 1.4:
                    continue

                # XpT [(2h k), q, n] = Wp_pair.T @ XT
                xpt_sb = b1pool.tile([128, PAIRS, N], BF16, tag="xpt",
                                     name="xpt_sb")
                for q in range(PAIRS):
                    for nh in range(2):
                        ps_xpt = psProj.tile([128, 512], F32, tag="proj",
                                             name="ps_xpt")
                        for t in range(DT):
                            nc.tensor.matmul(
                                ps_xpt,
                                wp_sb[:, t, 2 * q:2 * q + 2, :],
                                xt_sb[:, t, nh * 512:(nh + 1) * 512],
                                start=(t == 0), stop=(t == DT - 1))
                        if with_bias:
                            nc.vector.tensor_scalar_add(
                                ps_xpt, ps_xpt, bias_pcol[:, q:q + 1])
                        nc.vector.tensor_copy(
                            xpt_sb[:, q, nh * 512:(nh + 1) * 512], ps_xpt)

                if stage < 1.6:
                    continue

                # XcT [(j|0), h, n] = Cpad[h].T @ XpT  (zero rows kill the
                # other head's contribution; drains copy the zeros too)
                xct_sb = b1pool.tile([128, H, N], BF16, tag="xct", name="xct_sb")
                for h in range(H):
                    q = h // 2
                    for nh in range(2):
                        ps_xct = psProj.tile([128, 512], F32, tag="proj",
                                             name="ps_xct")
                        nc.tensor.matmul(ps_xct, ctpad[:, h, :],
                                         xpt_sb[:, q, nh * 512:(nh + 1) * 512],
                                         start=True, stop=True)
                        dst = xct_sb[:, h, nh * 512:(nh + 1) * 512]
                        if nh == 0:
                            nc.vector.tensor_copy(dst, ps_xct)
                        else:
                            nc.scalar.activation(dst, ps_xct, AF.Copy)

                out_sb = b1pool.tile([128, NCH, DOUT], F32, tag="outsb",
                                     name="out_sb")

                if stage < 3:
                    continue
                for q in range(PAIRS):
                    # scores + tanh + mask, per head
                    attn_t = []
                    for u in range(2):
                        h = 2 * q + u
                        at_u = attnp.tile([128, NCH, N], BF16, tag="attnT",
                                          name="attn_t")
                        for i in range(NCH):
                            ps_t = psT.tile([128, N], F32, tag="T", name="ps_t")
                            for nh in range(2):
                                nc.tensor.matmul(
                                    ps_t[:, nh * 512:(nh + 1) * 512],
                                    xpt_sb[:, q, i * 128:(i + 1) * 128],
                                    xct_sb[:, h, nh * 512:(nh + 1) * 512],
                                    start=True, stop=True)
                            nc.scalar.activation(at_u[:, i, :], ps_t, AF.Tanh)
                            eng = nc.gpsimd if i % 4 == 3 else nc.vector
                            eng.tensor_mul(at_u[:, i, :], at_u[:, i, :],
                                           at_sb[:, i, :])
                        attn_t.append(at_u)

                    if stage < 4:
                        continue
                    # aggregate: Xh[n, k] per j chunk, both heads -> relu -> out
                    for j in range(NCH):
                        ps_xh = psXh.tile([128, 2 * DH], F32, tag="xh",
                                          name="ps_xh")
                        for u in range(2):
                            h = 2 * q + u
                            for i in range(NCH):
                                nc.tensor.matmul(
                                    ps_xh[:, u * DH:(u + 1) * DH],
                                    attn_t[u][:, i, j * 128:(j + 1) * 128],
                                    xp_sb[:, i, h * DH:(h + 1) * DH],
                                    start=(i == 0), stop=(i == NCH - 1))
                        nc.vector.tensor_scalar_max(
                            out_sb[:, j, q * 128:(q + 1) * 128], ps_xh, 0.0)

                if stage >= 4:
                    for j in range(NCH):
                        nc.sync.dma_start(out=OUT[b, j * 128:(j + 1) * 128, :],
                                          in_=out_sb[:, j, :])

    nc.compile()
    return nc


_CACHED = {}


def _get_nc(with_bias: bool):
    if with_bias not in _CACHED:
        _CACHED[with_bias] = _build(with_bias)
    return _CACHED[with_bias]


def kernel(X, A, Wp, bp, C):
    global LAST_EXEC_NS, LAST_TRACE_DIR
    X = np.ascontiguousarray(np.asarray(X, dtype=np.float32))
    A = np.ascontiguousarray(np.asarray(A, dtype=np.float32))
    Wp = np.ascontiguousarray(np.asarray(Wp, dtype=np.float32))
    bp = np.ascontiguousarray(np.asarray(bp, dtype=np.float32))
    C = np.ascontiguousarray(np.asarray(C, dtype=np.float32))

    with_bias = bool(np.any(bp))
    nc = _get_nc(with_bias)

    in_maps = []
    for c in range(NCORES):
        m = {
            "X": X[c * BS:(c + 1) * BS],
            "A": A[c * BS:(c + 1) * BS],
            "Wp": Wp,
            "C": C,
        }
        if with_bias:
            m["bp"] = bp
        in_maps.append(m)

    trace = bool(int(os.environ.get("KERNEL_TRACE", "0")))
    res = run_bass_kernel_spmd(nc, in_maps, core_ids=list(range(NCORES)),
                               trace=trace)
    LAST_EXEC_NS = res.exec_time_ns
    if res.instructions_and_trace is not None:
        LAST_TRACE_DIR = res.instructions_and_trace[1]
    out = np.concatenate([res.results[c]["out"] for c in range(NCORES)], axis=0)
    return out.astype(np.float32)
